# revision 74
# baseline (speedup 1.0000x reference)
"""Linear-attention (relu, rmsnorm-qk) Trainium2 Bass kernel, 8 NeuronCores.

Sharding: each core owns 1/4 of the tokens of TWO batch elements:
  cores 0-3 -> batches 0 (group g=0) and 1 (g=1)
  cores 4-7 -> batches 2 (g=0) and 3 (g=1)
Within a batch, core q (= core_id % 4) owns tokens [1024*q, 1024*(q+1)).

v2 design notes (all-bf16 data path, fp32 PSUM):
 - q's rmsnorm scale s_q>0 cancels between attention numerator and
   denominator (relu(s*q) = s*relu(q), eps negligible), so q needs no
   stats at all and q^T is produced DIRECTLY via Wq^T @ x^T chunk
   matmuls (no PE transposes, no q-side DVE work; relu on ACT).
 - k's rmsnorm scale s_k is folded into v and into the appended
   "ones" column (which becomes s_t), so k needs only ACT relu and
   v gets the scale for free on its ACT copy-out.
 - kv_ext = k~^T @ [v~ | s] accumulated per head-pair in PSUM over
   2-tile batches, then DVE-added into SBUF; AllReduce over the 4
   cores of the batch overlaps the other group's phase 1.
 - phase 2: attn^T = blockdiag(kv)^T @ q^T, normalizer via replicated
   ksum columns, single DVE divide, out = attn @ W_out (+ b_out).
"""

import os
import sys

import numpy as np

for _p in ("/opt/trn_rl_repo",):
    if _p not in sys.path and os.path.isdir(_p):
        sys.path.insert(0, _p)

import concourse.mybir as mybir
import concourse.tile as tile
from concourse import bacc
from concourse.bass_utils import run_bass_kernel_spmd
from contextlib import ExitStack

F32 = mybir.dt.float32
BF16 = mybir.dt.bfloat16
ALU = mybir.AluOpType
ACTF = mybir.ActivationFunctionType

DIM = 1024
HEADS = 16
DHEAD = 64
NPAIR = HEADS // 2          # 8 head pairs
B = 4
N = 4096
TOK = 2048                  # tokens per core (2 groups x 1024)
GTOK = 1024                 # tokens per group
NTG = GTOK // 128           # 8 token tiles per group
EPS_NORM = 1e-6
KVW = 2 * (DHEAD + 1)       # 130: kv_ext width per pair
RG = [[0, 1, 2, 3], [4, 5, 6, 7]]

_CACHE: dict = {}


def _build(use_bias: bool, use_w: bool, sim_mode: bool = False, dbg: bool = False):
    ndev = 1 if sim_mode else 8
    nc = bacc.Bacc("TRN2", target_bir_lowering=False, debug=False, num_devices=ndev)

    xT_d = nc.dram_tensor("xT", [16, 128, DIM], BF16, kind="ExternalInput").ap()
    wqkv_d = nc.dram_tensor("wqkv", [8, 128, 3 * DIM], BF16, kind="ExternalInput").ap()
    wout_d = nc.dram_tensor("wout", [8, 128, DIM], BF16, kind="ExternalInput").ap()
    qn_d = nc.dram_tensor("qn", [128, 8], F32, kind="ExternalInput").ap()
    kn_d = nc.dram_tensor("kn", [128, DIM], F32, kind="ExternalInput").ap()
    bout_d = nc.dram_tensor("bout", [128, DIM], F32, kind="ExternalInput").ap()
    out_d = nc.dram_tensor("out", [TOK, DIM], F32, kind="ExternalOutput").ap()
    if dbg:
        dbg_qT = nc.dram_tensor(
            "dbg_qT", [8, 128, TOK], BF16, kind="ExternalOutput"
        ).ap()
        dbg_k = nc.dram_tensor(
            "dbg_k", [16, 128, DIM], BF16, kind="ExternalOutput"
        ).ap()
        dbg_v = nc.dram_tensor(
            "dbg_v", [16, 128, HEADS * (DHEAD + 1)], BF16, kind="ExternalOutput"
        ).ap()
        dbg_kvacc = nc.dram_tensor(
            "dbg_kvacc", [2, 128, NPAIR * KVW], F32, kind="ExternalOutput"
        ).ap()
        dbg_kvsb = nc.dram_tensor(
            "dbg_kvsb", [2, 128, NPAIR * KVW], F32, kind="ExternalOutput"
        ).ap()
        dbg_bd = nc.dram_tensor(
            "dbg_bd", [2, 128, NPAIR * 128], BF16, kind="ExternalOutput"
        ).ap()
        dbg_ksr = nc.dram_tensor(
            "dbg_ksr", [2, 128, NPAIR * 128], BF16, kind="ExternalOutput"
        ).ap()
        dbg_at = nc.dram_tensor(
            "dbg_at", [8, NPAIR, 128, 256], BF16, kind="ExternalOutput"
        ).ap()

    with tile.TileContext(nc) as tc:
        with ExitStack() as outer:
            const = outer.enter_context(tc.tile_pool(name="const", bufs=1))
            wpool = outer.enter_context(tc.tile_pool(name="wpool", bufs=1))
            qTpool = outer.enter_context(tc.tile_pool(name="qTpool", bufs=1))
            stats = outer.enter_context(tc.tile_pool(name="stats", bufs=3))
            drampool = outer.enter_context(
                tc.tile_pool(name="dram", bufs=1, space="DRAM")
            )

            eps_sb = const.tile([128, 1], F32, name="eps_sb")
            nc.vector.memset(eps_sb[:], EPS_NORM)
            ones_sb = const.tile([128, 64], F32, name="ones_sb")
            nc.vector.memset(ones_sb[:], 1.0)
            if use_w:
                qn_sb = const.tile([128, 8], F32, name="qn_sb")
                kn_sb = const.tile([128, DIM], F32, name="kn_sb")
                nc.sync.dma_start(qn_sb[:], qn_d[:])
                nc.sync.dma_start(kn_sb[:], kn_d[:])
            if use_bias:
                bout_sb = const.tile([128, DIM], F32, name="bout_sb")
                nc.sync.dma_start(bout_sb[:], bout_d[:])

            # x tiles stream on the ACT queue; tile 0 is issued before W so
            # its transfer leads the serial DMA stream.
            xTp = outer.enter_context(tc.tile_pool(name="xTp", bufs=3))
            xq = {}

            def issue_x(t):
                xt = xTp.tile([128, DIM], BF16, name=f"xT_{t}", tag="xT")
                nc.scalar.dma_start(xt[:], xT_d[t, :, :])
                xq[t] = xt

            issue_x(0)

            # W_qkv resident. The DMA engine pool drains transfers roughly
            # in issue order, so issue in first-use order: the q columns
            # (consumed by tile 0's q^T chains) before the k/v columns.
            w_sb = []
            for c in range(8):
                w = wpool.tile([128, 3 * DIM], BF16, name=f"wq{c}", tag=f"w{c}")
                w_sb.append(w)
                nc.sync.dma_start(w[:, 0:DIM], wqkv_d[c, :, 0:DIM])
            for c in range(8):
                nc.sync.dma_start(
                    w_sb[c][:, DIM : 3 * DIM], wqkv_d[c, :, DIM : 3 * DIM]
                )

            qT = [
                qTpool.tile([128, TOK], BF16, name=f"qT{j}", tag=f"qT{j}")
                for j in range(8)
            ]
            prep = outer.enter_context(tc.tile_pool(name="prep", bufs=1))

            def kv_prep(g, arout):
                # Entirely on the gpsimd queue: it is otherwise idle, its
                # FIFO orders the load after the collective, and it keeps
                # this prep off the busy DVE queue.
                kv_sb = prep.tile(
                    [128, NPAIR, KVW], F32, name=f"kvsb{g}", tag=f"kvsb{g}"
                )
                nc.gpsimd.dma_start(kv_sb[:], arout[:])
                if dbg:
                    nc.sync.dma_start(
                        dbg_kvsb[g, :, :],
                        kv_sb[:].rearrange("p a b -> p (a b)"),
                    )
                bd = prep.tile([128, NPAIR, 128], BF16, name=f"bd{g}", tag=f"bd{g}")
                nc.gpsimd.memset(bd[:], 0.0)
                nc.gpsimd.tensor_copy(bd[0:64, :, 0:64], kv_sb[0:64, :, 0:64])
                nc.gpsimd.tensor_copy(
                    bd[64:128, :, 64:128], kv_sb[64:128, :, 65:129]
                )
                ksr = prep.tile(
                    [128, NPAIR, 128], BF16, name=f"ksr{g}", tag=f"ksr{g}"
                )
                nc.gpsimd.memset(ksr[:], 0.0)
                for p in range(NPAIR):
                    nc.gpsimd.tensor_scalar_mul(
                        ksr[0:64, p, 0:64], ones_sb[0:64, 0:64],
                        kv_sb[0:64, p, 64:65],
                    )
                    nc.gpsimd.tensor_scalar_mul(
                        ksr[64:128, p, 64:128], ones_sb[64:128, 0:64],
                        kv_sb[64:128, p, 129:130],
                    )
                if dbg:
                    nc.sync.dma_start(
                        dbg_bd[g, :, :], bd[:].rearrange("p a b -> p (a b)")
                    )
                    nc.sync.dma_start(
                        dbg_ksr[g, :, :], ksr[:].rearrange("p a b -> p (a b)")
                    )
                return bd, ksr

            prepped = []
            with ExitStack() as ph1:
                kp = ph1.enter_context(tc.tile_pool(name="kp", bufs=3))
                vp = ph1.enter_context(tc.tile_pool(name="vp", bufs=3))
                kvpool = ph1.enter_context(tc.tile_pool(name="kvpool", bufs=2))
                psq = ph1.enter_context(
                    tc.tile_pool(name="psq", bufs=1, space="PSUM")
                )
                psk = ph1.enter_context(
                    tc.tile_pool(name="psk", bufs=1, space="PSUM")
                )
                psv = ph1.enter_context(
                    tc.tile_pool(name="psv", bufs=1, space="PSUM")
                )
                pssm = ph1.enter_context(
                    tc.tile_pool(name="pssm", bufs=1, space="PSUM")
                )

                def emit_qT(t, xt, eightbank):
                    t0r = t * 128
                    if eightbank:
                        # tile 0: all 8 banks are free, so run 8 chains
                        # c-major (one chain per bank — legal), consuming
                        # each W q-part the moment it lands
                        tags = ("q0", "q1", "k0", "k1", "v0", "v1", "sm0", "sm1")
                        pools = (psq, psq, psk, psk, psv, psv, pssm, pssm)
                        q8 = [
                            pools[j].tile(
                                [128, 128], F32, name=f"q8_{j}", tag=tags[j]
                            )
                            for j in range(8)
                        ]
                        for c in range(8):
                            for j in range(8):
                                nc.tensor.matmul(
                                    q8[j][:],
                                    w_sb[c][:, j * 128 : (j + 1) * 128],
                                    xt[:, c * 128 : (c + 1) * 128],
                                    start=(c == 0),
                                    stop=(c == 7),
                                )
                        for j in range(8):
                            nc.scalar.activation(
                                qT[j][:, t0r : t0r + 128],
                                q8[j][:],
                                ACTF.Relu,
                                scale=(qn_sb[:, j : j + 1] if use_w else 1.0),
                            )
                        return
                    qps = [
                        psq.tile(
                            [128, 4, 128], F32,
                            name=f"qps{t}_{half}", tag=f"q{half}",
                        )
                        for half in range(2)
                    ]
                    for half in range(2):
                        for jj in range(4):
                            j = 4 * half + jj
                            for c in range(8):
                                nc.tensor.matmul(
                                    qps[half][:, jj, :],
                                    w_sb[c][:, j * 128 : (j + 1) * 128],
                                    xt[:, c * 128 : (c + 1) * 128],
                                    start=(c == 0),
                                    stop=(c == 7),
                                )
                        for jj in range(4):
                            j = 4 * half + jj
                            nc.scalar.activation(
                                qT[j][:, t0r : t0r + 128],
                                qps[half][:, jj, :],
                                ACTF.Relu,
                                scale=(qn_sb[:, j : j + 1] if use_w else 1.0),
                            )

                def emit_kv_mm(t, xt, cmajor):
                    kps = [
                        psk.tile([128, 512], F32, name=f"kps{t}_{h}", tag=f"k{h}")
                        for h in range(2)
                    ]
                    vps = [
                        psv.tile([128, 512], F32, name=f"vps{t}_{h}", tag=f"v{h}")
                        for h in range(2)
                    ]

                    def kv_chunk(ps, base, h, c):
                        nc.tensor.matmul(
                            ps[h][:],
                            xt[:, c * 128 : (c + 1) * 128],
                            w_sb[c][:, base + h * 512 : base + (h + 1) * 512],
                            start=(c == 0),
                            stop=(c == 7),
                        )

                    if cmajor:
                        for c in range(8):
                            for h in range(2):
                                kv_chunk(kps, DIM, h, c)
                            for h in range(2):
                                kv_chunk(vps, 2 * DIM, h, c)
                    else:
                        for h in range(2):
                            for c in range(8):
                                kv_chunk(kps, DIM, h, c)
                        for h in range(2):
                            for c in range(8):
                                kv_chunk(vps, 2 * DIM, h, c)
                    return kps, vps

                def emit_epilogue(t, kps, vps):
                    # rmsnorm scale s = 1/sqrt(mean(k^2)+eps)
                    st6 = stats.tile([128, 2, 6], F32, name=f"st6_{t}", tag="st6")
                    nc.vector.bn_stats(st6[:, 0, :], kps[0][:])
                    nc.vector.bn_stats(st6[:, 1, :], kps[1][:])
                    mv = stats.tile([128, 2], F32, name=f"mv_{t}", tag="mv")
                    nc.vector.bn_aggr(mv[:], st6[:])
                    ms = stats.tile([128, 1], F32, name=f"ms_{t}", tag="ms")
                    nc.vector.scalar_tensor_tensor(
                        out=ms[:],
                        in0=mv[:, 0:1],
                        scalar=mv[:, 0:1],
                        in1=mv[:, 1:2],
                        op0=ALU.mult,
                        op1=ALU.add,
                    )
                    a0 = stats.tile([128, 1], F32, name=f"a0_{t}", tag="a0")
                    nc.scalar.activation(
                        a0[:], ms[:], ACTF.Sqrt, bias=eps_sb[:], scale=1.0
                    )
                    s = stats.tile([128, 1], F32, name=f"s_{t}", tag="s")
                    nc.vector.reciprocal(s[:], a0[:])

                    # k~ = relu(k) (scale folded into v); general path
                    # applies kn first on DVE.
                    k_sb = kp.tile([128, DIM], BF16, name=f"ksb{t}", tag="ksb")
                    for h in range(2):
                        sl = slice(h * 512, (h + 1) * 512)
                        if use_w:
                            nc.vector.tensor_tensor(
                                k_sb[:, sl], kps[h][:], kn_sb[:, sl], ALU.mult
                            )
                            nc.scalar.activation(
                                k_sb[:, sl], k_sb[:, sl], ACTF.Relu
                            )
                        else:
                            nc.scalar.activation(
                                k_sb[:, sl], kps[h][:], ACTF.Relu
                            )

                    v_sb = vp.tile(
                        [128, HEADS, DHEAD + 1], BF16, name=f"vsb{t}", tag="vsb"
                    )
                    for h in range(2):
                        nc.scalar.activation(
                            v_sb[:, 8 * h : 8 * (h + 1), 0:DHEAD],
                            vps[h].rearrange("p (h e) -> p h e", e=DHEAD),
                            ACTF.Copy,
                            scale=s[:],
                        )
                    nc.vector.tensor_scalar_mul(
                        v_sb[:, :, DHEAD], ones_sb[:, 0:16], s[:]
                    )
                    if dbg:
                        nc.sync.dma_start(dbg_k[t, :, :], k_sb[:])
                        nc.sync.dma_start(
                            dbg_v[t, :, :],
                            v_sb[:].rearrange("p h e -> p (h e)"),
                        )
                    return k_sb, v_sb

                def emit_kvbatch(i, t, sb0, sb1, kv_acc):
                    pk, pv = sb0
                    k_sb, v_sb = sb1
                    for grp, prs in ((0, (0, 1, 2)), (1, (3, 4, 5)), (2, (6, 7))):
                        kvp = pssm.tile(
                            [128, len(prs), KVW], F32,
                            name=f"kv{t}_{grp}",
                            tag=f"sm{0 if grp != 1 else 1}",
                        )
                        for pi, p in enumerate(prs):
                            for ii, (ks_, vs_) in enumerate(
                                ((pk, pv), (k_sb, v_sb))
                            ):
                                nc.tensor.matmul(
                                    kvp[:, pi, :],
                                    ks_[:, p * 128 : (p + 1) * 128],
                                    vs_[:, 2 * p : 2 * p + 2, :],
                                    start=(ii == 0),
                                    stop=(ii == 1),
                                )
                        if i == 1:
                            nc.vector.tensor_copy(
                                kv_acc[:, prs[0] : prs[-1] + 1, :], kvp[:]
                            )
                        else:
                            nc.vector.tensor_add(
                                kv_acc[:, prs[0] : prs[-1] + 1, :],
                                kv_acc[:, prs[0] : prs[-1] + 1, :],
                                kvp[:],
                            )

                for g in range(2):
                    kv_acc = kvpool.tile(
                        [128, NPAIR, KVW], F32, name=f"kvacc{g}", tag="kvacc"
                    )
                    if g > 0:
                        issue_x(8 * g)
                    issue_x(8 * g + 1)
                    hold = None
                    start_i = 0
                    if g == 0:
                        # prologue: both tiles' q^T (gated only on the early
                        # q-part stream) run before any k/v chain so the PE
                        # is never head-of-line blocked on late W slabs
                        xt0 = xq.pop(0)
                        xt1 = xq.pop(1)
                        emit_qT(0, xt0, eightbank=True)
                        issue_x(2)
                        emit_qT(1, xt1, eightbank=False)
                        kps0, vps0 = emit_kv_mm(0, xt0, cmajor=True)
                        sb0 = emit_epilogue(0, kps0, vps0)
                        issue_x(3)
                        kps1, vps1 = emit_kv_mm(1, xt1, cmajor=True)
                        sb1 = emit_epilogue(1, kps1, vps1)
                        emit_kvbatch(1, 1, sb0, sb1, kv_acc)
                        start_i = 2
                    for i in range(start_i, NTG):
                        t = 8 * g + i
                        if i < NTG - 2:
                            issue_x(t + 2)
                        xt = xq.pop(t)
                        emit_qT(t, xt, eightbank=False)
                        kps, vps = emit_kv_mm(t, xt, cmajor=False)
                        sb = emit_epilogue(t, kps, vps)
                        if i % 2 == 0:
                            hold = sb
                        else:
                            emit_kvbatch(i, t, hold, sb, kv_acc)
                            hold = None

                    if dbg:
                        nc.sync.dma_start(
                            dbg_kvacc[g, :, :],
                            kv_acc[:].rearrange("p a b -> p (a b)"),
                        )
                    arin = drampool.tile(
                        [128, NPAIR, KVW], F32, name=f"arin{g}", tag=f"arin{g}"
                    )
                    nc.sync.dma_start(arin[:], kv_acc[:])
                    arout = drampool.tile(
                        [128, NPAIR, KVW], F32, name=f"arout{g}", tag=f"arout{g}"
                    )
                    if sim_mode:
                        nc.sync.dma_start(arout[:], arin[:])
                    else:
                        nc.gpsimd.collective_compute(
                            "AllReduce",
                            ALU.add,
                            replica_groups=RG,
                            ins=[arin.opt()],
                            outs=[arout.opt()],
                        )
                    prepped.append(kv_prep(g, arout))
                    if g == 0:
                        wout_sb = []
                        for c in range(8):
                            w = wpool.tile(
                                [128, DIM], BF16, name=f"wo{c}", tag=f"wo{c}"
                            )
                            wout_sb.append(w)
                            nc.sync.dma_start(w[:], wout_d[c, :, :])

            # ------------- phase 2 -------------
            if dbg:
                for j in range(8):
                    nc.sync.dma_start(dbg_qT[j, :, :], qT[j][:])
            with ExitStack() as ph2:
                atp = ph2.enter_context(tc.tile_pool(name="atp", bufs=1))
                recp = ph2.enter_context(tc.tile_pool(name="recp", bufs=3))
                osbp = ph2.enter_context(tc.tile_pool(name="osbp", bufs=3))
                psattn = ph2.enter_context(
                    tc.tile_pool(name="psattn", bufs=2, space="PSUM")
                )
                psnorm = ph2.enter_context(
                    tc.tile_pool(name="psnorm", bufs=2, space="PSUM")
                )
                psout = ph2.enter_context(
                    tc.tile_pool(name="psout", bufs=4, space="PSUM")
                )

                for g in range(2):
                    bd, ksr = prepped[g]
                    for hc in range(4):
                        cc = 4 * g + hc
                        c0 = cc * 256
                        attnT = [
                            atp.tile(
                                [128, 256], BF16,
                                name=f"at{cc}_{p}", tag=f"at{p}_{cc % 2}",
                            )
                            for p in range(NPAIR)
                        ]
                        ops = [
                            [
                                psout.tile(
                                    [128, 512], F32, name=f"o{cc}_{tt}_{ff}",
                                    tag="ops",
                                )
                                for ff in range(2)
                            ]
                            for tt in range(2)
                        ]

                        def outproj_tt(tt, j):
                            lhsT = attnT[j][:, tt * 128 : (tt + 1) * 128]
                            for ff in range(2):
                                nc.tensor.matmul(
                                    ops[tt][ff][:],
                                    lhsT,
                                    wout_sb[j][:, ff * 512 : (ff + 1) * 512],
                                    start=(j == 0),
                                    stop=(j == 7),
                                )

                        def outproj_j(j):
                            for tt in range(2):
                                outproj_tt(tt, j)

                        for p in range(NPAIR):
                            aps = psattn.tile(
                                [128, 256], F32, name=f"aps{cc}_{p}", tag="aps"
                            )
                            nc.tensor.matmul(
                                aps[:], bd[:, p, :], qT[p][:, c0 : c0 + 256]
                            )
                            nps = psnorm.tile(
                                [128, 256], F32, name=f"nps{cc}_{p}", tag="nps"
                            )
                            nc.tensor.matmul(
                                nps[:], ksr[:, p, :], qT[p][:, c0 : c0 + 256]
                            )
                            rec = recp.tile(
                                [128, 256], F32, name=f"rec{cc}_{p}", tag="rec"
                            )
                            nc.vector.reciprocal_approx_fast(rec[:], nps[:])
                            nc.vector.tensor_tensor(
                                attnT[p][:], aps[:], rec[:], ALU.mult
                            )
                            if dbg:
                                nc.sync.dma_start(
                                    dbg_at[cc, p, :, :], attnT[p][:]
                                )
                            if cc == 7:
                                # stagger tt0 a step ahead so its output
                                # copies/DMAs overlap tt1's matmuls (tail)
                                if p >= 1:
                                    outproj_tt(0, p - 1)
                                if p >= 2:
                                    outproj_tt(1, p - 2)
                            elif p >= 1:
                                outproj_j(p - 1)
                        if cc == 7:
                            outproj_tt(0, 7)
                            outproj_tt(1, 6)
                            outproj_tt(1, 7)
                        else:
                            outproj_j(7)

                        for tt in range(2):
                            r0 = c0 + tt * 128
                            osb = osbp.tile(
                                [128, DIM], F32, name=f"osb{cc}{tt}", tag="osb"
                            )
                            for ff in range(2):
                                fsl = slice(ff * 512, (ff + 1) * 512)
                                if use_bias:
                                    nc.vector.tensor_tensor(
                                        osb[:, fsl], ops[tt][ff][:], bout_sb[:, fsl],
                                        ALU.add,
                                    )
                                elif cc == 7 and tt == 1 and ff == 1:
                                    nc.vector.tensor_copy(
                                        osb[:, fsl], ops[tt][ff][:]
                                    )
                                else:
                                    nc.scalar.copy(osb[:, fsl], ops[tt][ff][:])
                                (nc.sync if cc == 7 else nc.scalar).dma_start(
                                    out_d[r0 : r0 + 128, fsl], osb[:, fsl]
                                )

    nc.compile()
    return nc


def _get_nc(use_bias: bool, use_w: bool):
    key = ("nc", use_bias, use_w)
    if key not in _CACHE:
        _CACHE[key] = _build(use_bias, use_w)
    return _CACHE[key]


def make_in_maps(x, W_qkv, qn_w, kn_w, W_out, b_out):
    bf16 = mybir.dt.np(BF16)
    x = np.asarray(x, dtype=np.float32)
    W_qkv = np.ascontiguousarray(np.asarray(W_qkv, dtype=np.float32)).reshape(
        8, 128, 3 * DIM
    ).astype(bf16)
    W_out = np.ascontiguousarray(np.asarray(W_out, dtype=np.float32)).reshape(
        8, 128, DIM
    ).astype(bf16)
    qn = np.ascontiguousarray(
        np.asarray(qn_w, dtype=np.float32).reshape(8, 128).T
    )
    kn = np.ascontiguousarray(
        np.broadcast_to(np.asarray(kn_w, dtype=np.float32).reshape(1, DIM), (128, DIM))
    )
    bout = np.ascontiguousarray(
        np.broadcast_to(np.asarray(b_out, dtype=np.float32).reshape(1, DIM), (128, DIM))
    )
    in_maps = []
    for c in range(8):
        b0 = 2 * (c // 4)
        q = c % 4
        sl = slice(1024 * q, 1024 * (q + 1))
        xt = np.concatenate(
            [x[b0, sl, :].T, x[b0 + 1, sl, :].T], axis=1
        )  # [1024, 2048]
        # [16 tiles, 128 part (c-dims), 8 c-chunks x 128 tokens]
        xtt = np.ascontiguousarray(
            xt.reshape(8, 128, 16, 128).transpose(2, 1, 0, 3).reshape(16, 128, DIM)
        ).astype(bf16)
        in_maps.append(
            {
                "xT": xtt,
                "wqkv": W_qkv,
                "wout": W_out,
                "qn": qn,
                "kn": kn,
                "bout": bout,
            }
        )
    return in_maps


def assemble(results):
    out = np.empty((B, N, DIM), dtype=np.float32)
    for b in range(B):
        base = 4 * (b // 2)
        g = b % 2
        for q in range(4):
            out[b, 1024 * q : 1024 * (q + 1), :] = results[base + q]["out"][
                1024 * g : 1024 * (g + 1), :
            ]
    return out


def run(in_maps, use_bias, use_w, **kw):
    nc = _get_nc(use_bias, use_w)
    return run_bass_kernel_spmd(nc, in_maps, core_ids=list(range(8)), **kw)


def kernel(x, W_qkv, qn_w, kn_w, W_out, b_out):
    use_bias = bool(np.any(np.asarray(b_out)))
    use_w = not (
        np.all(np.asarray(qn_w) == 1.0) and np.all(np.asarray(kn_w) == 1.0)
    )
    in_maps = make_in_maps(x, W_qkv, qn_w, kn_w, W_out, b_out)
    res = run(in_maps, use_bias, use_w)
    return assemble(res.results)
